# revision 25
# baseline (speedup 1.0000x reference)
"""Trainium2 Bass kernel for nn_CausalPredictor_46462956208724.

Math: the reference computes
    wy = xm @ Wy_w.T + Wy_b            [L, 1]
    wz = dic_z @ Wz_w.T + Wz_b         [1, 1]
    attention = softmax(wy @ wz.T, axis=1)   # axis of size 1 -> exactly 1.0
    z = (attention * prior) @ dic_z    [L, C]
Softmax over a size-1 axis is exactly 1.0 in fp32 (exp(0)/exp(0) = 1/1),
so z[l, :] = prior[0] * dic[1, 0, :] for every row l, independent of xm.
The output is a broadcast of one scaled 1024-float row to 131072 rows —
a pure HBM-write problem (512 MB of output).

Sharding: pure data parallel over rows. 8 cores x 16384 rows each; the
tiny row + prior inputs are replicated to every core. Per core:
  1. DMA the 4 KB row into all 128 SBUF partitions (stride-0 DRAM-side
     partition dim), in two pipelined halves; scale by prior on DVE.
  2. Replicate to a [128, 16*1024] tile by DVE log-doubling copies.
  3. Store the 64 MB shard with stride-0-source DMAs. Output rows are
     mapped partition-contiguous (partition p <-> rows [p*128,(p+1)*128))
     so the bulk store uses 64 KB descriptors — smaller descriptors only
     sustain ~260 GB/s vs ~425-435 GB/s at 64 KB.
  4. Early small stores (2 MB + 6 MB) fill the window while the prefix
     is still replicating; they are placed so they never sit ahead of
     the bulk store in its HWDGE queue (queues drain FIFO).
Measured on hardware: ~173 us/core = 512 MB / ~3 TB/s chip-aggregate
HBM write bandwidth, with ~21 us of fixed head (engine boot + 4 KB row
fetch + replication) and ~2.4 us tail.
"""

import sys

for _p in (
    "/root/.axon_site",
    "/root/.axon_site/_ro/trn_rl_repo",
    "/root/.axon_site/_ro/pypackages",
    "/opt/trn_rl_repo",
):
    if _p not in sys.path:
        sys.path.append(_p)

import numpy as np

L = 131072
C = 1024
N_CORES = 8
SHARD = L // N_CORES          # 16384 rows per core
P = 128                       # SBUF partitions

_CACHE = {}


def _build_bass():
    import concourse.bacc as bacc
    import concourse.tile as tile
    from concourse import mybir

    f32 = mybir.dt.float32
    # Bacc (not raw Bass): its compile() pipeline splits multi-sem waits
    # into event semaphores — TRN2 allows at most 1 wait per instruction,
    # and walrus rejects the raw IR with "Too many sync wait commands".
    nc = bacc.Bacc(None)
    row_in = nc.declare_dram_parameter("row", [1, C], f32, isOutput=False)
    prior_in = nc.declare_dram_parameter("prior", [1, 1], f32, isOutput=False)
    out = nc.declare_dram_parameter("out", [SHARD, C], f32, isOutput=True)

    with tile.TileContext(nc) as tc:
        with tc.tile_pool(name="pool", bufs=1) as pool:
            # Stride-0 partition dim on the DRAM side: every SBUF
            # partition receives the same row/scalar in one normal
            # 128-partition DMA (a [1, N] DMA would be sprayed across
            # all 16 queues, and a consumer waiting on that many queue
            # semaphores overflows the 1-wait-per-instruction limit).
            # Two halves so the scale-multiply overlaps the second DMA.
            h = C // 2
            col = pool.tile([P, C], f32)
            prb = pool.tile([P, 1], f32)
            nc.sync.dma_start(
                out=col[:, 0:h], in_=row_in[:, 0:h].partition_broadcast(P)
            )
            nc.sync.dma_start(
                out=col[:, h:C], in_=row_in[:, h:C].partition_broadcast(P)
            )
            nc.scalar.dma_start(out=prb[:], in_=prior_in[:].partition_broadcast(P))

            # Same-engine copy so the multiply carries one sem wait and
            # FIFO-orders after it (cheaper than an event-sem split).
            prb2 = pool.tile([P, 1], f32)
            nc.vector.tensor_copy(prb2[:], prb[:])

            # big holds the scaled row replicated 16x per partition.
            # Replication is DVE-only: concurrent GpSimd SBUF traffic
            # degrades DVE copies ~7x (shared SBUF ports).
            big = pool.tile([P, 16 * C], f32)
            out_pc = out[:].rearrange("(p r) c -> p r c", p=P)
            nc.vector.tensor_mul(
                big[:, 0:h], col[:, 0:h], prb2[:].broadcast_to([P, h])
            )
            nc.vector.tensor_mul(
                big[:, h:C], col[:, h:C], prb2[:].broadcast_to([P, h])
            )
            nc.vector.tensor_copy(big[:, C : 2 * C], big[:, 0:C])
            # A: rows r[0:2) — 1 MB on sync; sized to drain before the
            # bulk store C needs the queue.
            nc.sync.dma_start(out=out_pc[:, 0:2, :], in_=big[:, 0 : 2 * C])
            # B1: rows r[2:4) — 1 MB on scalar.
            nc.scalar.dma_start(out=out_pc[:, 2:4, :], in_=big[:, 0 : 2 * C])
            nc.vector.tensor_copy(big[:, 2 * C : 4 * C], big[:, 0 : 2 * C])
            # B2: rows r[4:16) from the 4C prefix — 6 MB on scalar, which
            # carries no 64 KB traffic this kernel.
            nc.scalar.dma_start(
                out=out_pc[:, 4:16, :],
                in_=big[:, 0 : 4 * C].unsqueeze(1).broadcast_to([P, 3, 4 * C]),
            )
            nc.vector.tensor_copy(big[:, 4 * C : 8 * C], big[:, 0 : 4 * C])
            nc.vector.tensor_copy(big[:, 8 * C : 16 * C], big[:, 0 : 8 * C])
            # C: rows r[16:128) — 56 MB of 64 KB descriptors on sync.
            nc.sync.dma_start(
                out=out_pc[:, 16:128, :],
                in_=big[:, 0 : 16 * C].unsqueeze(1).broadcast_to([P, 7, 16 * C]),
            )
    nc.compile()
    return nc


def _get_nc():
    if "nc" not in _CACHE:
        _CACHE["nc"] = _build_bass()
    return _CACHE["nc"]


def kernel(x, xm, Wy_w, Wy_b, Wz_w, Wz_b, dic, prior, **_unused):
    from concourse.bass_utils import run_bass_kernel_spmd

    nc = _get_nc()
    row = np.ascontiguousarray(np.asarray(dic, dtype=np.float32)[1].reshape(1, C))
    pr = np.ascontiguousarray(np.asarray(prior, dtype=np.float32).reshape(1, 1))
    in_maps = [{"row": row, "prior": pr} for _ in range(N_CORES)]
    res = run_bass_kernel_spmd(nc, in_maps, list(range(N_CORES)))
    shards = [res.results[i]["out"] for i in range(N_CORES)]
    full = np.concatenate(shards, axis=0).reshape(L, 1, C)
    return full
